# revision 2
# baseline (speedup 1.0000x reference)
"""EdgeGAT Trainium2 kernel: 3-layer GAT + BN + edge MLP + edge scorer.

Strategy (edge-parallel, dst-sorted):
- Pad nodes to NP=50176=8*6272; core k owns dst range [k*6272,(k+1)*6272).
- Edges (incl. self loops) sorted by dst, sharded by dst range, grouped into
  128-node windows, padded to 128-edge chunks (lo/hi split for int16 gather).
- Per layer: phase A computes H=[h@W | ls | ld] tables (replicated dense
  matmuls from the transposed h table); phase B gathers H[src] rows with
  dma_gather, builds p=exp(leakyrelu(ls+ld)) per edge, aggregates
  sum_e p*h[src] and sum_e p per dst via selection-matrix matmuls accumulated
  in PSUM per window, normalizes by 1/s at window end.
- elu+BN applied on the transposed own slice; stats all-reduced; the
  normalized transposed slice all-gathered to become the next layer's input.
- Final: per-node u/v = h3 @ [ws|wd] on device, edge MLP dot on device; host
  assembles out = u[src0] + v[dst0] + qef.
"""
import numpy as np

import concourse.bass as bass
import concourse.mybir as mybir
import concourse.tile as tile
from concourse import bacc
from concourse.bass_utils import run_bass_kernel_spmd
from concourse.masks import make_identity

FP = mybir.dt.float32
I16 = mybir.dt.int16
ALU = mybir.AluOpType
ACTF = mybir.ActivationFunctionType

NCORE = 8
P = 128
N = 50000
E0 = 1600000
NP = 50176          # padded node count (8 * 6272)
NR = NP // NCORE    # 6272 nodes per core
WPC = NR // P       # 49 windows per core
F_IN = 128
HID = 64
SLOPE = 0.2
EPS_BN = 1e-5
GK = 4              # chunks per dma_gather call
LO = 32768          # int16 split point for src gather
EQ = ((E0 // NCORE + 511) // 512) * 512   # padded qef edges per core (200192)
SENT = -1000.0      # ld sentinel for padded edges -> p = exp(<= -200) = 0

# layer configs: (Din, heads, Dout, Hrow width, ls col offset in H row)
LAYERS = [
    (128, 4, 256, 320, 256),
    (256, 4, 256, 320, 256),
    (256, 1, 64, 128, 64),
]


def _wrap16(idx):
    """[n] int array -> [128, ceil(n/16)] int16 (idx i at [i%16, i//16], 16-row
    block replicated 8x down partitions)."""
    n = len(idx)
    S = (n + 15) // 16
    a = np.zeros((16, S), dtype=np.int16)
    a[np.arange(n) % 16, np.arange(n) // 16] = idx.astype(np.int16)
    return np.tile(a, (8, 1))


def _groups(cl):
    """chunk count -> list of group sizes (each <= GK)."""
    out = []
    r = cl
    while r > 0:
        g = min(GK, r)
        out.append(g)
        r -= g
    return out


def build_kernel(CLlo, CLhi, np_nodes=NP, nr=NR, eq=EQ):
    """CLlo/CLhi: per-window-slot chunk counts (len WPC lists)."""
    wpc = nr // P
    nc = bacc.Bacc("TRN2", target_bir_lowering=False, debug=False,
                   num_devices=NCORE)

    # ---------------- inputs ----------------
    xT_in = nc.dram_tensor("xT", [P, np_nodes], FP, kind="ExternalInput")
    xTown_in = nc.dram_tensor("xTown", [P, nr], FP, kind="ExternalInput")
    wext1_in = nc.dram_tensor("wext1", [128, 320], FP, kind="ExternalInput")
    wext2_in = nc.dram_tensor("wext2", [256, 320], FP, kind="ExternalInput")
    wext3_in = nc.dram_tensor("wext3", [256, 128], FP, kind="ExternalInput")
    wld1_in = nc.dram_tensor("wld1", [128, 64], FP, kind="ExternalInput")
    wld2_in = nc.dram_tensor("wld2", [256, 64], FP, kind="ExternalInput")
    wld3_in = nc.dram_tensor("wld3", [256, 64], FP, kind="ExternalInput")
    wsd_in = nc.dram_tensor("wsd", [64, 2], FP, kind="ExternalInput")
    qvec_in = nc.dram_tensor("qvec", [64, 1], FP, kind="ExternalInput")
    mlpw1_in = nc.dram_tensor("mlpw1", [16, 64], FP, kind="ExternalInput")
    b1e_in = nc.dram_tensor("b1e", [64, 1], FP, kind="ExternalInput")
    cconst_in = nc.dram_tensor("cconst", [1, 1], FP, kind="ExternalInput")
    bng_in = nc.dram_tensor("bng", [2, 2, 128, 1], FP, kind="ExternalInput")
    bnb_in = nc.dram_tensor("bnb", [2, 2, 128, 1], FP, kind="ExternalInput")
    brep1_in = nc.dram_tensor("brep1", [128, 256], FP, kind="ExternalInput")
    brep2_in = nc.dram_tensor("brep2", [128, 256], FP, kind="ExternalInput")
    brep3_in = nc.dram_tensor("brep3", [128, 64], FP, kind="ExternalInput")
    eaT_in = nc.dram_tensor("eaT", [16, eq], FP, kind="ExternalInput")

    lo_cols = sum(cl * 8 for cl in CLlo)
    hi_cols = sum(cl * 8 for cl in CLhi)
    ct_list = [a + b for a, b in zip(CLlo, CLhi)]
    ld_cols = sum(ct * 8 for ct in ct_list)
    dl_cols = sum(ct_list)
    idxlo_in = nc.dram_tensor("idxlo", [P, max(lo_cols, 1)], I16, kind="ExternalInput")
    idxhi_in = nc.dram_tensor("idxhi", [P, max(hi_cols, 1)], I16, kind="ExternalInput")
    idxld_in = nc.dram_tensor("idxld", [P, ld_cols], I16, kind="ExternalInput")
    dstloc_in = nc.dram_tensor("dstloc", [P, dl_cols], FP, kind="ExternalInput")

    uv_out = nc.dram_tensor("uv", [nr, 2], FP, kind="ExternalOutput")
    qef_out = nc.dram_tensor("qef", [1, eq], FP, kind="ExternalOutput")

    with tile.TileContext(nc) as tc:
        with (
            tc.tile_pool(name="const", bufs=1) as cpool,
            tc.tile_pool(name="wpool", bufs=1) as wpool,
            tc.tile_pool(name="dram", bufs=1, space="DRAM") as dpool,
        ):
            # constants
            ident = cpool.tile([P, P], FP)
            make_identity(nc, ident[:])
            iota_i = cpool.tile([P, P], mybir.dt.int32)
            nc.gpsimd.iota(iota_i[:], pattern=[[1, P]], base=0, channel_multiplier=0)
            iotaF = cpool.tile([P, P], FP)
            nc.vector.tensor_copy(out=iotaF[:], in_=iota_i[:])

            # persistent weights in SBUF
            def load_w(dram, shape, tag):
                t = wpool.tile(shape, FP, tag=tag)
                nc.sync.dma_start(out=t[:], in_=dram[:])
                return t
            wext1 = load_w(wext1_in, [128, 320], "wx1")
            wext2a = wpool.tile([128, 320], FP)
            nc.sync.dma_start(out=wext2a[:], in_=wext2_in[0:128, :])
            wext2b = wpool.tile([128, 320], FP)
            nc.sync.dma_start(out=wext2b[:], in_=wext2_in[128:256, :])
            wext3a = wpool.tile([128, 128], FP)
            nc.sync.dma_start(out=wext3a[:], in_=wext3_in[0:128, :])
            wext3b = wpool.tile([128, 128], FP)
            nc.sync.dma_start(out=wext3b[:], in_=wext3_in[128:256, :])
            wld1 = load_w(wld1_in, [128, 64], "wl1")
            wld2a = wpool.tile([128, 64], FP)
            nc.sync.dma_start(out=wld2a[:], in_=wld2_in[0:128, :])
            wld2b = wpool.tile([128, 64], FP)
            nc.sync.dma_start(out=wld2b[:], in_=wld2_in[128:256, :])
            wld3a = wpool.tile([128, 64], FP)
            nc.sync.dma_start(out=wld3a[:], in_=wld3_in[0:128, :])
            wld3b = wpool.tile([128, 64], FP)
            nc.sync.dma_start(out=wld3b[:], in_=wld3_in[128:256, :])
            wsd = load_w(wsd_in, [64, 2], "wsd")
            qvec = load_w(qvec_in, [64, 1], "qv")
            mlpw1 = load_w(mlpw1_in, [16, 64], "mw1")
            b1e = load_w(b1e_in, [64, 1], "b1e")
            cconst = load_w(cconst_in, [1, 1], "cc")
            brep1 = load_w(brep1_in, [128, 256], "br1")
            brep2 = load_w(brep2_in, [128, 256], "br2")
            brep3 = load_w(brep3_in, [128, 64], "br3")
            brep = [brep1, brep2, brep3]
            sent = cpool.tile([P, 64], FP)
            nc.gpsimd.memset(sent[:], SENT)

            # DRAM scratch
            H12 = dpool.tile([np_nodes, 320], FP)     # layer 1/2 H table (reused)
            H3 = dpool.tile([np_nodes, 128], FP)
            LDW = dpool.tile([6400 if nr == 6272 else nr + 128, 64], FP)
            hTloc = dpool.tile([2, P, nr], FP)        # pre-elu own transposed slice
            agin = [dpool.tile([2, P, nr], FP, name=f"agin{i}", tag=f"agin{i}") for i in range(2)]
            agout = [dpool.tile([NCORE, 2, P, nr], FP, name=f"agout{i}", tag=f"agout{i}") for i in range(2)]
            stats_in = [dpool.tile([2, P, 2], FP, name=f"stats_in{i}", tag=f"stats_in{i}") for i in range(2)]
            stats_out = [dpool.tile([2, P, 2], FP, name=f"stats_out{i}", tag=f"stats_out{i}") for i in range(2)]

            ntile = np_nodes // P

            def phase_a(li):
                """dense: H table + (own) LDW rows for layer li (0-based)."""
                Din, NH, Dout, HW, LS0 = LAYERS[li]
                kf = Din // 128
                Htab = H12 if li < 2 else H3
                wext = [wext1] if li == 0 else ([wext2a, wext2b] if li == 1 else [wext3a, wext3b])
                wld = [wld1] if li == 0 else ([wld2a, wld2b] if li == 1 else [wld3a, wld3b])
                with (
                    tc.tile_pool(name=f"pa{li}", bufs=4) as pa,
                    tc.tile_pool(name=f"pap{li}", bufs=4, space="PSUM") as pap,
                ):
                    for t in range(ntile):
                        ps = pap.tile([P, HW], FP, tag="ps", space="PSUM")
                        for f in range(kf):
                            lt = pa.tile([P, P], FP, tag="lt")
                            if li == 0:
                                nc.sync.dma_start(out=lt[:], in_=xT_in[:, t * P:(t + 1) * P])
                            else:
                                r, tw = t // wpc, t % wpc
                                nc.sync.dma_start(
                                    out=lt[:],
                                    in_=agout[li - 1][r, f, :, tw * P:(tw + 1) * P])
                            nc.tensor.matmul(out=ps[:], lhsT=lt[:], rhs=wext[f][:],
                                             start=(f == 0), stop=(f == kf - 1))
                        hx = pa.tile([P, HW], FP, tag="hx")
                        if t % 2 == 0:
                            nc.vector.tensor_copy(out=hx[:], in_=ps[:])
                        else:
                            nc.scalar.activation(out=hx[:], in_=ps[:], func=ACTF.Copy)
                        nc.sync.dma_start(out=Htab[t * P:(t + 1) * P, :], in_=hx[:])
                    # own-slice LD table (ls|ld|pad, 64 cols) + sentinel rows
                    for w in range(wpc):
                        ps = pap.tile([P, 64], FP, tag="psld", space="PSUM")
                        for f in range(kf):
                            lt = pa.tile([P, P], FP, tag="lt")
                            if li == 0:
                                nc.sync.dma_start(out=lt[:], in_=xTown_in[:, w * P:(w + 1) * P])
                            else:
                                nc.sync.dma_start(
                                    out=lt[:], in_=agin[li - 1][f, :, w * P:(w + 1) * P])
                            nc.tensor.matmul(out=ps[:], lhsT=lt[:], rhs=wld[f][:],
                                             start=(f == 0), stop=(f == kf - 1))
                        lx = pa.tile([P, 64], FP, tag="lx")
                        nc.vector.tensor_copy(out=lx[:], in_=ps[:])
                        nc.sync.dma_start(out=LDW[w * P:(w + 1) * P, :], in_=lx[:])
                    nc.sync.dma_start(out=LDW[nr:nr + P, :], in_=sent[:])

            import os as _os
            KB = int(_os.environ.get("KB", "4"))

            def phase_b(li):
                """edge phase for layer li: gather, p, aggregate, normalize."""
                Din, NH, Dout, HW, LS0 = LAYERS[li]
                Htab = H12 if li < 2 else H3
                ldsl = (4, 4 + NH) if li < 2 else (1, 1 + NH)
                lo_off = [0]
                hi_off = [0]
                ld_off = [0]
                dl_off = [0]
                for w in range(wpc):
                    lo_off.append(lo_off[-1] + CLlo[w] * 8)
                    hi_off.append(hi_off[-1] + CLhi[w] * 8)
                    ld_off.append(ld_off[-1] + ct_list[w] * 8)
                    dl_off.append(dl_off[-1] + ct_list[w])
                with (
                    tc.tile_pool(name=f"pb{li}", bufs=3) as pb,
                    tc.tile_pool(name=f"pbs{li}", bufs=4) as pbs,
                    tc.tile_pool(name=f"pbp{li}", bufs=2, space="PSUM") as pbp,
                    tc.tile_pool(name=f"pbp2{li}", bufs=2, space="PSUM") as pbp2,
                ):
                    for w in range(wpc):
                        ct = ct_list[w]
                        # per-window streams
                        dloc = pbs.tile([P, ct], FP, tag="dloc")
                        nc.sync.dma_start(out=dloc[:], in_=dstloc_in[:, dl_off[w]:dl_off[w] + ct])
                        ldidx = pbs.tile([P, ct * 8], I16, tag="ldidx")
                        nc.sync.dma_start(out=ldidx[:], in_=idxld_in[:, ld_off[w]:ld_off[w] + ct * 8])
                        ldg = pbs.tile([P, ct, 64], FP, tag="ldg")
                        for seg in range(0, ct, GK):
                            sgk = min(GK, ct - seg)
                            nc.gpsimd.dma_gather(
                                out_ap=ldg[:, seg:seg + sgk, :], in_ap=LDW[:],
                                idxs_ap=ldidx[:, seg * 8:(seg + sgk) * 8],
                                num_idxs=sgk * P, num_idxs_reg=sgk * P, elem_size=64)
                        p_win = pbs.tile([P, ct, NH], FP, tag="pwin")
                        # gather groups: lo then hi chunk slots
                        hg_tiles = []
                        calls = ([("lo", c0, g) for c0, g in _iter_groups(CLlo[w])] +
                                 [("hi", c0, g) for c0, g in _iter_groups(CLhi[w])])
                        for half, c0, gk in calls:
                            base = (lo_off[w] if half == "lo" else hi_off[w]) + c0 * 8
                            idxs = pb.tile([P, GK * 8], I16, tag="gidx")
                            src_arr = idxlo_in if half == "lo" else idxhi_in
                            nc.sync.dma_start(out=idxs[:, :gk * 8],
                                              in_=src_arr[:, base:base + gk * 8])
                            hg = pb.tile([P, GK, HW], FP, tag="hg")
                            tbl = Htab[0:LO, :] if half == "lo" else Htab[LO:np_nodes, :]
                            nc.gpsimd.dma_gather(
                                out_ap=hg[:, :gk, :], in_ap=tbl, idxs_ap=idxs[:, :gk * 8],
                                num_idxs=gk * P, num_idxs_reg=gk * P, elem_size=HW)
                            cbase = c0 if half == "lo" else CLlo[w] + c0
                            hg_tiles.append((cbase, gk, hg))
                            if KB < 2:
                                continue
                            # dense p ops for this group
                            lgt = pb.tile([P, GK, NH], FP, tag="lgt")
                            nc.vector.tensor_tensor(
                                out=lgt[:, :gk, :], in0=hg[:, :gk, LS0:LS0 + NH],
                                in1=ldg[:, cbase:cbase + gk, ldsl[0]:ldsl[1]], op=ALU.add)
                            t2 = pb.tile([P, GK, NH], FP, tag="t2")
                            nc.vector.tensor_scalar_mul(t2[:, :gk, :], lgt[:, :gk, :], SLOPE)
                            nc.vector.tensor_tensor(out=lgt[:, :gk, :], in0=lgt[:, :gk, :],
                                                    in1=t2[:, :gk, :], op=ALU.max)
                            nc.scalar.activation(out=p_win[:, cbase:cbase + gk, :],
                                                 in_=lgt[:, :gk, :], func=ACTF.Exp)
                        if KB < 2:
                            continue
                        psum_o = pbp.tile([P, Dout], FP, tag="po", space="PSUM")
                        psum_s = pbp2.tile([P, NH], FP, tag="psm", space="PSUM")
                        for cbase, gk, hg in (hg_tiles if KB >= 3 else []):
                            for j in range(gk):
                                c = cbase + j
                                S = pb.tile([P, P], FP, tag="S")
                                nc.vector.tensor_tensor(
                                    out=S[:], in0=dloc[:, c:c + 1].to_broadcast([P, P]),
                                    in1=iotaF[:], op=ALU.is_equal)
                                msg = pb.tile([P, Dout], FP, tag="msg")
                                for h in range(NH):
                                    sl = slice(h * HID, (h + 1) * HID)
                                    if h % 2 == 0:
                                        nc.vector.tensor_scalar_mul(
                                            msg[:, sl], hg[:, j, sl], p_win[:, c, h:h + 1])
                                    else:
                                        nc.scalar.activation(
                                            out=msg[:, sl], in_=hg[:, j, sl],
                                            func=ACTF.Copy, scale=p_win[:, c, h:h + 1])
                                first, last = (c == 0), (c == ct - 1)
                                nc.tensor.matmul(out=psum_o[:], lhsT=S[:], rhs=msg[:],
                                                 start=first, stop=last)
                                nc.tensor.matmul(out=psum_s[:], lhsT=S[:],
                                                 rhs=p_win[:, c, :], start=first, stop=last)
                        if KB < 4:
                            continue
                        # window end: normalize, bias, transpose/store
                        s_sb = pbs.tile([P, NH], FP, tag="ssb")
                        nc.vector.tensor_scalar_add(s_sb[:], psum_s[:], 1e-12)
                        rs = pbs.tile([P, NH], FP, tag="rs")
                        nc.vector.reciprocal(out=rs[:], in_=s_sb[:])
                        hwin = pbs.tile([P, Dout], FP, tag="hwin")
                        for h in range(NH):
                            sl = slice(h * HID, (h + 1) * HID)
                            if h % 2 == 0:
                                nc.scalar.activation(out=hwin[:, sl], in_=psum_o[:, sl],
                                                     func=ACTF.Copy, scale=rs[:, h:h + 1])
                            else:
                                nc.vector.tensor_scalar_mul(hwin[:, sl], psum_o[:, sl],
                                                            rs[:, h:h + 1])
                        nc.vector.tensor_tensor(out=hwin[:], in0=hwin[:], in1=brep[li][:],
                                                op=ALU.add)
                        if li < 2:
                            for f in range(2):
                                tp = pbp.tile([P, P], FP, tag="tp", space="PSUM")
                                nc.tensor.transpose(out=tp[:], in_=hwin[:, f * P:(f + 1) * P],
                                                    identity=ident[:])
                                tsb = pbs.tile([P, P], FP, tag="tsb")
                                if f == 0:
                                    nc.vector.tensor_copy(out=tsb[:], in_=tp[:])
                                else:
                                    nc.scalar.activation(out=tsb[:], in_=tp[:], func=ACTF.Copy)
                                nc.sync.dma_start(out=hTloc[f, :, w * P:(w + 1) * P], in_=tsb[:])
                        else:
                            tp = pbp.tile([P, P], FP, tag="tp", space="PSUM")
                            nc.tensor.transpose(out=tp[:64, :], in_=hwin[:, 0:64],
                                                identity=ident[:])
                            tsb = pbs.tile([64, P], FP, tag="tsb3")
                            nc.vector.tensor_copy(out=tsb[:], in_=tp[:64, :])
                            uvp = pbp2.tile([P, 2], FP, tag="uvp", space="PSUM")
                            nc.tensor.matmul(out=uvp[:], lhsT=tsb[:], rhs=wsd[:],
                                             start=True, stop=True)
                            uvs = pbs.tile([P, 2], FP, tag="uvs")
                            nc.vector.tensor_copy(out=uvs[:], in_=uvp[:])
                            nc.sync.dma_start(out=uv_out[w * P:(w + 1) * P, :], in_=uvs[:])

            def elu_bn_ag(li):
                """li in {0,1}: elu + BN on own transposed slice, then AllGather."""
                sizes = []
                off = 0
                while off < nr:
                    sz = min(512, nr - off)
                    sizes.append((off, sz))
                    off += sz
                nchunk = len(sizes)
                with (
                    tc.tile_pool(name=f"pe{li}", bufs=4) as pe,
                    tc.tile_pool(name=f"pes{li}", bufs=1) as pes,
                ):
                    sums = [pes.tile([P, nchunk], FP, name=f"sm{f}", tag=f"sm{f}") for f in range(2)]
                    sqs = [pes.tile([P, nchunk], FP, name=f"sq{f}", tag=f"sq{f}") for f in range(2)]
                    for f in range(2):
                        for i, (off, sz) in enumerate(sizes):
                            xt = pe.tile([P, 512], FP, tag="xt")
                            nc.sync.dma_start(out=xt[:, :sz], in_=hTloc[f, :, off:off + sz])
                            r = pe.tile([P, 512], FP, tag="r")
                            nc.scalar.activation(out=r[:, :sz], in_=xt[:, :sz], func=ACTF.Relu)
                            m = pe.tile([P, 512], FP, tag="m")
                            nc.vector.tensor_scalar_min(m[:, :sz], xt[:, :sz], 0.0)
                            e = pe.tile([P, 512], FP, tag="e")
                            nc.scalar.activation(out=e[:, :sz], in_=m[:, :sz], func=ACTF.Exp)
                            y = pe.tile([P, 512], FP, tag="y")
                            nc.vector.tensor_tensor(out=y[:, :sz], in0=r[:, :sz],
                                                    in1=e[:, :sz], op=ALU.add)
                            nc.vector.tensor_scalar_add(y[:, :sz], y[:, :sz], -1.0)
                            nc.vector.tensor_reduce(out=sums[f][:, i:i + 1], in_=y[:, :sz],
                                                    axis=mybir.AxisListType.X, op=ALU.add)
                            sq = pe.tile([P, 512], FP, tag="sq")
                            nc.scalar.activation(out=sq[:, :sz], in_=y[:, :sz], func=ACTF.Square)
                            nc.vector.tensor_reduce(out=sqs[f][:, i:i + 1], in_=sq[:, :sz],
                                                    axis=mybir.AxisListType.X, op=ALU.add)
                            nc.sync.dma_start(out=agin[li][f, :, off:off + sz], in_=y[:, :sz])
                        st = pe.tile([P, 2], FP, tag="st")
                        nc.vector.tensor_reduce(out=st[:, 0:1], in_=sums[f][:],
                                                axis=mybir.AxisListType.X, op=ALU.add)
                        nc.vector.tensor_reduce(out=st[:, 1:2], in_=sqs[f][:],
                                                axis=mybir.AxisListType.X, op=ALU.add)
                        nc.sync.dma_start(out=stats_in[li][f], in_=st[:])
                    nc.gpsimd.collective_compute(
                        "AllReduce", ALU.add,
                        ins=[stats_in[li].opt()], outs=[stats_out[li].opt()],
                        replica_groups=[list(range(NCORE))])
                    for f in range(2):
                        st = pe.tile([P, 2], FP, tag="st2")
                        nc.sync.dma_start(out=st[:], in_=stats_out[li][f])
                        gt = pe.tile([P, 1], FP, tag="gt")
                        nc.sync.dma_start(out=gt[:], in_=bng_in[li, f])
                        bt = pe.tile([P, 1], FP, tag="bt")
                        nc.sync.dma_start(out=bt[:], in_=bnb_in[li, f])
                        mu = pe.tile([P, 1], FP, tag="mu")
                        nc.vector.tensor_scalar_mul(mu[:], st[:, 0:1], 1.0 / N)
                        msq = pe.tile([P, 1], FP, tag="msq")
                        nc.vector.tensor_scalar_mul(msq[:], st[:, 1:2], 1.0 / N)
                        mu2 = pe.tile([P, 1], FP, tag="mu2")
                        nc.scalar.activation(out=mu2[:], in_=mu[:], func=ACTF.Square)
                        var = pe.tile([P, 1], FP, tag="var")
                        nc.vector.tensor_tensor(out=var[:], in0=msq[:], in1=mu2[:],
                                                op=ALU.subtract)
                        nc.vector.tensor_scalar_add(var[:], var[:], EPS_BN)
                        sd = pe.tile([P, 1], FP, tag="sd")
                        nc.scalar.activation(out=sd[:], in_=var[:], func=ACTF.Sqrt)
                        rstd = pe.tile([P, 1], FP, tag="rstd")
                        nc.vector.reciprocal(out=rstd[:], in_=sd[:])
                        scale = pe.tile([P, 1], FP, tag="scale")
                        nc.vector.tensor_tensor(out=scale[:], in0=rstd[:], in1=gt[:],
                                                op=ALU.mult)
                        mscale = pe.tile([P, 1], FP, tag="mscale")
                        nc.vector.tensor_tensor(out=mscale[:], in0=mu[:], in1=scale[:],
                                                op=ALU.mult)
                        shift = pe.tile([P, 1], FP, tag="shift")
                        nc.vector.tensor_tensor(out=shift[:], in0=bt[:], in1=mscale[:],
                                                op=ALU.subtract)
                        for (off, sz) in sizes:
                            yt = pe.tile([P, 512], FP, tag="yt")
                            nc.sync.dma_start(out=yt[:, :sz], in_=agin[li][f, :, off:off + sz])
                            nc.vector.tensor_scalar(
                                out=yt[:, :sz], in0=yt[:, :sz], scalar1=scale[:],
                                scalar2=shift[:], op0=ALU.mult, op1=ALU.add)
                            nc.sync.dma_start(out=agin[li][f, :, off:off + sz], in_=yt[:, :sz])
                    nc.gpsimd.collective_compute(
                        "AllGather", ALU.bypass,
                        ins=[agin[li].opt()], outs=[agout[li].opt()],
                        replica_groups=[list(range(NCORE))])

            def qef_phase():
                with (
                    tc.tile_pool(name="pq", bufs=4) as pq,
                    tc.tile_pool(name="pqp", bufs=4, space="PSUM") as pqp,
                ):
                    for t in range(eq // 512):
                        ea = pq.tile([16, 512], FP, tag="ea")
                        nc.sync.dma_start(out=ea[:], in_=eaT_in[:, t * 512:(t + 1) * 512])
                        mp = pqp.tile([64, 512], FP, tag="mp", space="PSUM")
                        nc.tensor.matmul(out=mp[:], lhsT=mlpw1[:], rhs=ea[:],
                                         start=True, stop=True)
                        msb = pq.tile([64, 512], FP, tag="msb")
                        nc.scalar.activation(out=msb[:], in_=mp[:], func=ACTF.Relu,
                                             bias=b1e[:])
                        qp = pqp.tile([1, 512], FP, tag="qp", space="PSUM")
                        nc.tensor.matmul(out=qp[:], lhsT=qvec[:], rhs=msb[:],
                                         start=True, stop=True)
                        qsb = pq.tile([1, 512], FP, tag="qsb")
                        nc.scalar.activation(out=qsb[:], in_=qp[:], func=ACTF.Identity,
                                             bias=cconst[0:1, :])
                        nc.sync.dma_start(out=qef_out[0:1, t * 512:(t + 1) * 512], in_=qsb[:])

            # ---------------- schedule ----------------
            import os
            stage = int(os.environ.get("KSTAGE", "9"))
            # 0=weights only, 1=qef, 2=+A1, 3=+B1, 4=+elubn1, 9=full
            if stage >= 9:
                for li in range(3):
                    phase_a(li)
                    phase_b(li)
                    if li < 2:
                        elu_bn_ag(li)
                qef_phase()
            else:
                if stage >= 1:
                    qef_phase()
                if stage >= 2:
                    phase_a(0)
                if stage >= 3:
                    phase_b(0)
                if stage >= 4:
                    elu_bn_ag(0)

    nc.compile()
    return nc


def _iter_groups(cl):
    out = []
    c0 = 0
    for g in _groups(cl):
        out.append((c0, g))
        c0 += g
    return out


# ===================== host side =====================

def _host_prep(x, edge_index, edge_attr, W1, a1s, a1d, b1, W2, a2s, a2d, b2,
               W3, a3s, a3d, b3, bn1_g, bn1_b, bn2_g, bn2_b,
               mlp_W1, mlp_b1, mlp_W2, mlp_b2, fc_W, fc_b):
    f32 = np.float32
    x = np.asarray(x, f32)
    ei = np.asarray(edge_index)
    ea = np.asarray(edge_attr, f32)

    def blockdiag(a):  # [H, D] -> [H*D, H]
        Hh, D = a.shape
        out = np.zeros((Hh * D, Hh), f32)
        for h in range(Hh):
            out[h * D:(h + 1) * D, h] = a[h]
        return out

    def wext(W, a_s, a_d, width):
        Din, Dout = W.shape
        Hh = a_s.shape[0]
        out = np.zeros((Din, width), f32)
        out[:, :Dout] = W
        out[:, Dout:Dout + Hh] = W @ blockdiag(a_s)
        out[:, Dout + Hh:Dout + 2 * Hh] = W @ blockdiag(a_d)
        return out

    def wld(W, a_s, a_d, lscols):
        Din = W.shape[0]
        Hh = a_s.shape[0]
        out = np.zeros((Din, 64), f32)
        out[:, 0:Hh] = W @ blockdiag(a_s)
        out[:, lscols:lscols + Hh] = W @ blockdiag(a_d)
        return out

    weights = {
        "wext1": wext(np.asarray(W1, f32), np.asarray(a1s, f32), np.asarray(a1d, f32), 320),
        "wext2": wext(np.asarray(W2, f32), np.asarray(a2s, f32), np.asarray(a2d, f32), 320),
        "wext3": wext(np.asarray(W3, f32), np.asarray(a3s, f32), np.asarray(a3d, f32), 128),
        "wld1": wld(np.asarray(W1, f32), np.asarray(a1s, f32), np.asarray(a1d, f32), 4),
        "wld2": wld(np.asarray(W2, f32), np.asarray(a2s, f32), np.asarray(a2d, f32), 4),
        "wld3": wld(np.asarray(W3, f32), np.asarray(a3s, f32), np.asarray(a3d, f32), 1),
        "wsd": np.stack([np.asarray(fc_W, f32)[0:64, 0],
                         np.asarray(fc_W, f32)[64:128, 0]], axis=1).copy(),
        "qvec": (np.asarray(mlp_W2, f32) @ np.asarray(fc_W, f32)[128:192, 0])[:, None].copy(),
        "mlpw1": np.asarray(mlp_W1, f32).copy(),
        "b1e": np.asarray(mlp_b1, f32)[:, None].copy(),
        "cconst": np.array([[float(np.asarray(mlp_b2, f32) @ np.asarray(fc_W, f32)[128:192, 0]
                                   + np.asarray(fc_b, f32)[0])]], f32),
        "bng": np.stack([np.asarray(bn1_g, f32).reshape(2, 128, 1),
                         np.asarray(bn2_g, f32).reshape(2, 128, 1)]),
        "bnb": np.stack([np.asarray(bn1_b, f32).reshape(2, 128, 1),
                         np.asarray(bn2_b, f32).reshape(2, 128, 1)]),
        "brep1": np.tile(np.asarray(b1, f32), (128, 1)),
        "brep2": np.tile(np.asarray(b2, f32), (128, 1)),
        "brep3": np.tile(np.asarray(b3, f32), (128, 1)),
    }

    xT = np.zeros((128, NP), f32)
    xT[:, :N] = x.T

    # edges with self loops, sorted by dst
    src0 = ei[0].astype(np.int64)
    dst0 = ei[1].astype(np.int64)
    loops = np.arange(N, dtype=np.int64)
    src = np.concatenate([src0, loops])
    dst = np.concatenate([dst0, loops])
    order = np.argsort(dst, kind="stable")
    ssrc = src[order]
    sdst = dst[order]
    core = sdst // NR
    win = (sdst % NR) // P

    # per (core, window, half) edge lists
    lo_mask = ssrc < LO
    lists = {}
    for c in range(NCORE):
        cm = core == c
        for w in range(WPC):
            wm = cm & (win == w)
            lists[(c, w, 0)] = np.nonzero(wm & lo_mask)[0]
            lists[(c, w, 1)] = np.nonzero(wm & ~lo_mask)[0]

    CLlo = [max(1, max((len(lists[(c, w, 0)]) + P - 1) // P for c in range(NCORE)))
            for w in range(WPC)]
    CLhi = [max(1, max((len(lists[(c, w, 1)]) + P - 1) // P for c in range(NCORE)))
            for w in range(WPC)]

    per_core = []
    for c in range(NCORE):
        idxlo_blocks, idxhi_blocks, ldid_blocks, dloc_blocks = [], [], [], []
        for w in range(WPC):
            e_lo = lists[(c, w, 0)]
            e_hi = lists[(c, w, 1)]
            nlo, nhi = CLlo[w] * P, CLhi[w] * P
            # per-half padded per-edge arrays
            gsrc_lo = np.zeros(nlo, np.int64)
            gsrc_lo[:len(e_lo)] = ssrc[e_lo]
            gsrc_hi = np.zeros(nhi, np.int64)
            gsrc_hi[:len(e_hi)] = ssrc[e_hi] - LO
            ldrow = np.full(nlo + nhi, NR, np.int64)   # sentinel row
            ldrow[:len(e_lo)] = sdst[e_lo] - c * NR
            ldrow[nlo:nlo + len(e_hi)] = sdst[e_hi] - c * NR
            dloc = np.zeros(nlo + nhi, f32)
            dloc[:len(e_lo)] = (sdst[e_lo] - c * NR - w * P).astype(f32)
            dloc[nlo:nlo + len(e_hi)] = (sdst[e_hi] - c * NR - w * P).astype(f32)
            # wrap idx per gather call
            for c0, g in _iter_groups(CLlo[w]):
                idxlo_blocks.append(_wrap16(gsrc_lo[c0 * P:(c0 + g) * P]))
            for c0, g in _iter_groups(CLhi[w]):
                idxhi_blocks.append(_wrap16(gsrc_hi[c0 * P:(c0 + g) * P]))
            ldid_blocks.append(_wrap16(ldrow))
            ct = CLlo[w] + CLhi[w]
            dloc_blocks.append(dloc.reshape(ct, P).T.copy())   # [p, c]
        d = {
            "idxlo": np.concatenate(idxlo_blocks, axis=1),
            "idxhi": np.concatenate(idxhi_blocks, axis=1),
            "idxld": np.concatenate(ldid_blocks, axis=1),
            "dstloc": np.concatenate(dloc_blocks, axis=1),
            "xTown": xT[:, c * NR:(c + 1) * NR].copy(),
        }
        # qef inputs: original-order edge slice
        e0, e1 = c * (E0 // NCORE), (c + 1) * (E0 // NCORE)
        eaT = np.zeros((16, EQ), f32)
        eaT[:, :e1 - e0] = ea[e0:e1].T
        d["eaT"] = eaT
        per_core.append(d)

    return weights, xT, per_core, CLlo, CLhi, src0, dst0


LAST_EXEC_NS = None
LAST_RES = None


def kernel(**inputs):
    global LAST_EXEC_NS, LAST_RES
    weights, xT, per_core, CLlo, CLhi, src0, dst0 = _host_prep(**inputs)
    nc = build_kernel(CLlo, CLhi)
    in_maps = []
    for c in range(NCORE):
        m = dict(weights)
        m["xT"] = xT
        m.update(per_core[c])
        in_maps.append(m)
    import os as _os2
    _trace = bool(int(_os2.environ.get("KTRACE", "0")))
    res = run_bass_kernel_spmd(nc, in_maps, core_ids=list(range(NCORE)),
                               trace=_trace)
    LAST_RES = res
    LAST_EXEC_NS = res.exec_time_ns
    uv = np.concatenate([res.results[c]["uv"] for c in range(NCORE)], axis=0)
    qef = np.concatenate([res.results[c]["qef"][0, :E0 // NCORE] for c in range(NCORE)])
    out = uv[src0, 0] + uv[dst0, 1] + qef
    return out.astype(np.float32)

